# revision 9
# baseline (speedup 1.0000x reference)
"""Causal self-attention (softmax over the QUERY axis) for Trainium2, 8 cores.

Reference semantics (B=2, S=2048, D=1024, H=16, HD=64):
    q = x @ Wq; k = x @ Wk; v = x @ Wv          (per batch)
    s[b,h,q,k] = <q_bqh, k_bkh>;  mask k > q -> -inf
    w = softmax(s / sqrt(1024), axis=q)          # normalize over QUERY axis
    ctx[b,q,h,:] = sum_k w[b,h,q,k] * v[b,k,h,:]

Sharding: core c handles batch b = c // 4 and head group g = c % 4
(4 heads: 4g..4g+3).  Per core everything is done in a transposed
score layout S^T[k, q], which makes the query-axis softmax a FREE-AXIS
reduction, and the 1/Z[k] normalizer folds into V rows (no per-element
divide): ctx[q,d] = sum_k exp(s)/Z[k] * v[k,d] = sum_k exp(s) * (v[k,d]/Z[k]).

Key structure (v2, ACT-engine-centric):
  - Causal diag masking is done IN PSUM via one extra matmul per score
    row: I128^T @ TRI adds -1e6 to the strictly-lower part of the 128x128
    diagonal block, so exp() produces exact zeros and the row sum (Z) is
    correct with no post-hoc correction (no gpsimd selects, no inv sums).
  - Score rows ping-pong between a 4-bank [128,2048] and a 3-bank
    [128,1536] PSUM tile, so each row is ONE activation instruction;
    short rows (kt>=8) are packed in pairs into one activation.
  - Z: accum_out on solo rows (kt 0..7), DVE post-zero row-reduce for
    packed rows (kt 8..15).
  - exp() is the only real work on the Scalar queue (input DMAs moved to
    gpsimd/vector queues); E is stored per head as one packed [128,17408]
    bf16 tile (row kt at col E_OFF[kt]).
  - A short burst of dummy matmuls during the input-DMA window pre-warms
    the PE HAM clock gate so real matmuls start at 2.4 GHz.

Device layouts (per core):
    xT  [1024, 2048] bf16 (host-transposed)  -> SBUF [128, 8, 2048]
    Wq/Wk/Wv column slices [1024, 256] bf16  -> SBUF [128, 8, 256]
    qT/kT  [128(2 heads x 64), 2 pairs, 2048] bf16 (projection output)
    v      [128(s in tile), 16 kt, 256(4 heads x 64)] bf16 (scaled by 1/Z in place)
    E      per head [128, 17408] bf16, row kt at cols [E_OFF[kt], +2048-128kt)
    out    [256(4 heads x 64), 2048] f32 = ctx^T; host transposes back.
"""

import numpy as np
import ml_dtypes
from contextlib import ExitStack

import concourse.bass as bass
import concourse.tile as tile
from concourse import bacc, mybir
from concourse.bass_utils import run_bass_kernel_spmd

BF16 = mybir.dt.bfloat16
F32 = mybir.dt.float32

B, S, D, H, HD = 2, 2048, 1024, 16, 64
NCORES = 8
HL = 4                       # heads per core
KC = D // 128                # 8 contraction chunks
KT = S // 128                # 16 key tiles
QC = S // 512                # 4 query chunks of 512
SCALE = 1.0 / float(np.sqrt(np.float32(D)))   # 1/32
MASK_BIG = -1.0e6

W_ROW = [S - 128 * kt for kt in range(KT)]          # valid width of E row kt
E_OFF = np.concatenate([[0], np.cumsum(W_ROW)]).astype(int)
E_TOT = int(E_OFF[-1])                              # 17408

# score-row emission plan per query chunk (descending heads 0..2):
#   each entry: (rows_tuple, which_psum_tile)  'A' = [128,2048], 'B' = [128,1536]
QC_GROUPS = {
    3: [((14, 15), 'B'), ((12, 13), 'A')],
    2: [((10, 11), 'B'), ((8, 9), 'A')],
    1: [((7,), 'B'), ((6,), 'A'), ((5,), 'B'), ((4,), 'A')],
    0: [((3,), 'A'), ((2,), 'A'), ((1,), 'A'), ((0,), 'A')],
}
# head 3 runs ascending in groups of 4 rows so pair-1 ctx can start early
G_GROUPS = {
    0: [((0,), 'A'), ((1,), 'A'), ((2,), 'A'), ((3,), 'A')],
    1: [((4,), 'B'), ((5,), 'A'), ((6,), 'B'), ((7,), 'A')],
    2: [((8, 9), 'A'), ((10, 11), 'B')],
    3: [((12, 13), 'A'), ((14, 15), 'B')],
}


def _emit(ctx: ExitStack, tc: tile.TileContext, out_ap, xT, wq, wk, wv, ident, tri):
    nc = tc.nc
    Exp = mybir.ActivationFunctionType.Exp

    consts = ctx.enter_context(tc.tile_pool(name="consts", bufs=1))
    qkp = ctx.enter_context(tc.tile_pool(name="qk", bufs=1))
    vp = ctx.enter_context(tc.tile_pool(name="v", bufs=1))
    epool = ctx.enter_context(tc.tile_pool(name="e", bufs=3))
    zpool = ctx.enter_context(tc.tile_pool(name="z", bufs=4))
    outp = ctx.enter_context(tc.tile_pool(name="outp", bufs=1))
    # PSUM: 4-bank + 3-bank score tiles (ping-pong) + one bank for proj/ctx
    psA = ctx.enter_context(tc.tile_pool(name="psA", bufs=1, space="PSUM"))
    psB = ctx.enter_context(tc.tile_pool(name="psB", bufs=1, space="PSUM"))
    small_ps = ctx.enter_context(tc.tile_pool(name="small_ps", bufs=1, space="PSUM"))

    # ---- input DMAs: both HWDGE rings (sync + scalar).  The two scalar-ring
    # issues happen before any activation work exists, so the ACT queue is
    # free again well before the first exp ----
    xT_r = xT.rearrange("(c p) s -> p c s", p=128)
    xT_cs = [None] * 4

    def load_chunk(sc, eng):
        xT_cs[sc] = consts.tile([128, KC, 512], BF16, tag=f"xT{sc}",
                                name=f"xT{sc}_sb")
        eng.dma_start(out=xT_cs[sc], in_=xT_r[:, :, 512 * sc:512 * sc + 512])

    w_sb = {}

    def load_w(name, t):
        w_sb[name] = consts.tile([128, KC, HL * HD], BF16, tag=f"w{name}",
                                 name=f"w{name}_sb")
        nc.sync.dma_start(out=w_sb[name], in_=t.rearrange("(c p) n -> p c n", p=128))

    load_chunk(2, nc.scalar)
    load_chunk(0, nc.scalar)
    load_w("q", wq)
    load_w("k", wk)
    load_chunk(3, nc.sync)
    ident_sb = consts.tile([128, 128], BF16, tag="ident", name="ident_sb")
    nc.sync.dma_start(out=ident_sb, in_=ident)
    tri_sb = consts.tile([128, 128], BF16, tag="tri", name="tri_sb")
    nc.sync.dma_start(out=tri_sb, in_=tri)
    load_chunk(1, nc.sync)
    load_w("v", wv)

    def xT_slice(c, lo, w):
        sc, o = divmod(lo, 512)
        assert o + w <= 512
        return xT_cs[sc][:, c, o:o + w]

    qT_sb = qkp.tile([128, 2, S], BF16, tag="qT")
    kT_sb = qkp.tile([128, 2, S], BF16, tag="kT")
    v_sb = vp.tile([128, KT, HL * HD], BF16, tag="v")
    out_sb = outp.tile([128, 2, S], F32, tag="out")
    sp_tile = small_ps.tile([128, 512], F32, tag="ps", name="sp")
    sp_par = [0]

    def next_sp():
        s = sp_tile[:, 256 * sp_par[0]:256 * sp_par[0] + 256]
        sp_par[0] ^= 1
        return s

    # ---- PE warm-up: dummy matmuls during the DMA window so HAM reaches
    # K=8/8 before the first projection chain (8 disjoint regions so no
    # write-after-write sync gets inserted between them) ----
    warm = consts.tile([128, 256], BF16, tag="warm", name="warm_sb")
    nc.vector.memset(warm, 0.0)
    wps = psA.tile([128, 2048], F32, tag="sA", name="warmps")
    for i in range(24):
        r = 256 * (i % 8)
        nc.tensor.matmul(wps[:, r:r + 256], warm[:, 0:128], warm,
                         start=True, stop=True)

    def proj_chain(name, pair, qc, half):
        dst = qT_sb if name == "q" else kT_sb
        ps = next_sp()
        for c in range(KC):
            nc.tensor.matmul(
                ps,
                w_sb[name][:, c, 128 * pair:128 * pair + 128],
                xT_cs[qc][:, c, 256 * half:256 * half + 256],
                start=(c == 0), stop=(c == KC - 1),
            )
        lo = 512 * qc + 256 * half
        nc.vector.tensor_copy(dst[:, pair, lo:lo + 256], ps)

    def proj_v():
        # v natural layout: out partitions = s-within-tile, cols = 4 heads x 64
        for st in range(KT):
            ps = next_sp()
            for c in range(KC):
                nc.tensor.matmul(
                    ps,
                    xT_slice(c, 128 * st, 128),
                    w_sb["v"][:, c, :],
                    start=(c == 0), stop=(c == KC - 1),
                )
            nc.vector.tensor_copy(v_sb[:, st, :], ps)

    def alloc_head(h):
        return {
            "h": h,
            "E": epool.tile([128, E_TOT], BF16, tag="E", name=f"E{h}"),
            "zp": zpool.tile([128, KT], F32, tag="zp", name=f"zp{h}"),
            "zi": zpool.tile([128, KT], F32, tag="zi", name=f"zi{h}"),
        }

    def score_group(st, rows, which):
        """Matmul rows into one PSUM tile (with in-PSUM causal mask via the
        I^T @ TRI trick), one exp() activation, then per-row Z."""
        h = st["h"]
        pair, half = divmod(h, 2)
        pb = 64 * half
        offs = []
        o = 0
        for kt in rows:
            offs.append(o)
            o += W_ROW[kt]
        wtot = o
        if which == 'A':
            ps = psA.tile([128, 2048], F32, tag="sA", name="sA")
        else:
            ps = psB.tile([128, 1536], F32, tag="sB", name="sB")
        for kt, po in zip(rows, offs):
            q0k = 128 * kt
            W = W_ROW[kt]
            lhsT = kT_sb[pb:pb + 64, pair, q0k:q0k + 128]   # [64, 128]
            # chunks split at PSUM 512 (bank) boundaries
            c = po
            first = True
            while c < po + W:
                c1 = min(po + W, (c // 512 + 1) * 512)
                nc.tensor.matmul(
                    ps[:, c:c1],
                    lhsT,
                    qT_sb[pb:pb + 64, pair, q0k + c - po:q0k + c1 - po],
                    start=True, stop=not first,
                )
                first = False
                c = c1
            # causal mask: add -1e6 on the strictly-lower part of the
            # 128x128 diagonal block (q < k) so exp() yields exact zeros
            nc.tensor.matmul(
                ps[:, po:po + 128], ident_sb, tri_sb,
                start=False, stop=True,
            )
        e_dst = st["E"][:, int(E_OFF[rows[0]]):int(E_OFF[rows[0]]) + wtot]
        if len(rows) == 1:
            kt = rows[0]
            nc.scalar.activation(e_dst, ps[:, 0:wtot], Exp, scale=SCALE,
                                 accum_out=st["zp"][:, kt:kt + 1])
        else:
            nc.scalar.activation(e_dst, ps[:, 0:wtot], Exp, scale=SCALE)
            for kt in rows:
                nc.vector.tensor_reduce(
                    st["zp"][:, kt:kt + 1],
                    st["E"][:, int(E_OFF[kt]):int(E_OFF[kt]) + W_ROW[kt]],
                    axis=mybir.AxisListType.X, op=mybir.AluOpType.add,
                )

    def v2_scale(st, k0, k1):
        """finalize 1/Z for rows [k0, k1) and scale this head's V cols."""
        h = st["h"]
        nc.vector.reciprocal(st["zi"][:, k0:k1], st["zp"][:, k0:k1])
        zia = st["zi"][:, k0:k1]
        zi_bc = bass.AP(tensor=zia.tensor, offset=zia.offset,
                        ap=[zia.ap[0], zia.ap[1], [0, HD]])
        nc.vector.tensor_mul(
            v_sb[:, k0:k1, HD * h:HD * h + HD],
            v_sb[:, k0:k1, HD * h:HD * h + HD],
            zi_bc,
        )

    def ctx_pair(sta, stb, qc):
        """col-packed ctx chains for a whole pair (heads sta, stb) at qc,
        in two 256-col halves so the psum tiles double-buffer in 1 bank."""
        pair = sta["h"] // 2
        for h256 in (0, 1):
            lo_q = 512 * qc + 256 * h256
            ps = next_sp()
            n_kt = 4 * qc + 2 * h256 + 2
            for kt in range(n_kt):
                q0 = max(lo_q, 128 * kt)
                w = lo_q + 256 - q0
                for half, st in ((0, sta), (1, stb)):
                    h = st["h"]
                    lo = int(E_OFF[kt]) + q0 - 128 * kt
                    nc.tensor.matmul(
                        ps[64 * half:64 * half + 64, q0 - lo_q:256],
                        v_sb[:, kt, HD * h:HD * h + HD],
                        st["E"][:, lo:lo + w],
                        start=(kt == 0), stop=(kt == n_kt - 1),
                        tile_position=(0, 64 * half),
                        skip_group_check=True,
                    )
            nc.vector.tensor_copy(out_sb[:, pair, lo_q:lo_q + 256], ps)

    def out_dma(pair, qc):
        nc.sync.dma_start(
            out=out_ap[128 * pair:128 * pair + 128, 512 * qc:512 * qc + 512],
            in_=out_sb[:, pair, 512 * qc:512 * qc + 512],
        )

    # ---- emission (order = scheduling priority) ----
    st0 = alloc_head(0)
    for qc in (3, 2, 1, 0):           # head 0 interleaved with its projections
        for half in (0, 1):
            proj_chain("q", 0, qc, half)
        for half in (0, 1):
            proj_chain("k", 0, qc, half)
        for rows, which in QC_GROUPS[qc]:
            score_group(st0, rows, which)
    st1 = alloc_head(1)
    for qc in (3, 2, 1, 0):           # head 1: projections already done
        for rows, which in QC_GROUPS[qc]:
            score_group(st1, rows, which)
    proj_v()                          # PE filler while ACT chews heads 0-1
    v2_scale(st0, 0, KT)
    v2_scale(st1, 0, KT)
    for qc in (3, 2, 1, 0):           # pair-1 projections: filler
        for half in (0, 1):
            proj_chain("q", 1, qc, half)
        for half in (0, 1):
            proj_chain("k", 1, qc, half)
    st2 = alloc_head(2)
    for qc in (3, 2, 1, 0):
        for rows, which in QC_GROUPS[qc]:
            score_group(st2, rows, which)
    for g in range(4):                # pair-0 ctx: PE filler during head-2 exp
        ctx_pair(st0, st1, g)
        out_dma(0, g)
    v2_scale(st2, 0, KT)
    st3 = alloc_head(3)
    for g in range(4):                # head 3 ascending; pair-1 ctx follows
        for rows, which in G_GROUPS[g]:
            score_group(st3, rows, which)
        v2_scale(st3, 4 * g, 4 * g + 4)
        ctx_pair(st2, st3, g)
        out_dma(1, g)


_PROG = None


def _build_program():
    global _PROG
    if _PROG is not None:
        return _PROG
    nc = bacc.Bacc("TRN2", target_bir_lowering=False, debug=False,
                   num_devices=NCORES)
    xT = nc.dram_tensor("xT", [D, S], BF16, kind="ExternalInput").ap()
    wq = nc.dram_tensor("wq", [D, HL * HD], BF16, kind="ExternalInput").ap()
    wk = nc.dram_tensor("wk", [D, HL * HD], BF16, kind="ExternalInput").ap()
    wv = nc.dram_tensor("wv", [D, HL * HD], BF16, kind="ExternalInput").ap()
    ident = nc.dram_tensor("ident", [128, 128], BF16, kind="ExternalInput").ap()
    tri = nc.dram_tensor("tri", [128, 128], BF16, kind="ExternalInput").ap()
    out = nc.dram_tensor("out", [HL * HD, S], F32, kind="ExternalOutput").ap()
    with tile.TileContext(nc) as tc:
        with ExitStack() as stack:
            _emit(stack, tc, out, xT, wq, wk, wv, ident, tri)
    nc.compile()
    _PROG = nc
    return nc


def make_in_maps(x, Wq, Wk, Wv):
    bf = ml_dtypes.bfloat16
    ident = np.eye(128, dtype=bf)
    tri = np.tril(np.full((128, 128), MASK_BIG, np.float32), -1).astype(bf)
    in_maps = []
    for core in range(NCORES):
        b, g = divmod(core, NCORES // B)
        cols = slice(HL * HD * g, HL * HD * (g + 1))
        in_maps.append({
            "xT": np.ascontiguousarray(np.asarray(x[b]).T).astype(bf),
            "wq": np.ascontiguousarray(np.asarray(Wq)[:, cols]).astype(bf),
            "wk": np.ascontiguousarray(np.asarray(Wk)[:, cols]).astype(bf),
            "wv": np.ascontiguousarray(np.asarray(Wv)[:, cols]).astype(bf),
            "ident": ident,
            "tri": tri,
        })
    return in_maps


def assemble(results):
    out = np.empty((B, S, H * HD), np.float32)
    for core in range(NCORES):
        b, g = divmod(core, NCORES // B)
        out[b, :, HL * HD * g:HL * HD * (g + 1)] = results[core]["out"].T
    return out


def kernel(**inputs):
    nc = _build_program()
    in_maps = make_in_maps(inputs["x"], inputs["Wq"], inputs["Wk"], inputs["Wv"])
    res = run_bass_kernel_spmd(nc, in_maps, list(range(NCORES)))
    return assemble(res.results)
